# revision 7
# baseline (speedup 1.0000x reference)
"""Trainium2 Bass kernel for nn_DR2FWL2ConvSimple (DR2FWL2 GNN conv layer).

Strategy (8-way SPMD, zero collectives):
  * Every segment-sum stream is sorted by target index on the host and
    sharded by target range, so all scatter-adds are core-local.
  * The permutation term lin(u + u[inv1]) is rewritten as two segment-sums
    (targets t0 and argsort(inv1)[t0]) of the same per-triangle stream.
  * On device, gathers are 1024-row indirect DMAs from bf16 tables; per-row
    work: PE transpose -> inner linear (bf16 matmul, f32 PSUM) -> ReLU ->
    one-hot matmul scatter into equal-row PSUM windows -> per-window post
    linears -> C-major assembly + 2-layer MLP -> f32 residual add.
  * The program is identical across cores (uniform window/tile counts via
    host padding); only per-core input tensors differ.
"""

import os
import sys

for _p in ("/opt/trn_rl_repo", "/root/.axon_site/_ro/trn_rl_repo"):
    if os.path.isdir(_p) and _p not in sys.path:
        sys.path.append(_p)

from contextlib import ExitStack

import ml_dtypes
import numpy as np

import concourse.bass as bass
import concourse.tile as tile
from concourse import mybir
from concourse.bass_utils import run_bass_kernel_spmd
from concourse.vector_clock import ScopedClock

F32 = mybir.dt.float32
BF16 = mybir.dt.bfloat16
I32 = mybir.dt.int32
BF = ml_dtypes.bfloat16

NCORES = 8
C = 128
PW = 256          # rows (and max bins) per window
GW = 4            # windows per gather/IO group
PAD_TGT = -1.0e6  # one-hot-miss sentinel for padded scatter rows

RELU = mybir.ActivationFunctionType.Relu
COPY = mybir.ActivationFunctionType.Copy
ADD = mybir.AluOpType.add
ISEQ = mybir.AluOpType.is_equal

NSWQ = 4  # SWDGE queues for indirect-gather descriptor-gen parallelism
_swq_counter = [0]


def _gather_tiles(nc, out_tile, table_ap, idx_ap, cols):
    """Gather 128 rows per offset column: HW indirect DMA consumes exactly one
    offset per partition, so emit one op per 128-row tile, round-robined
    across SWDGE queues."""
    for j in range(cols):
        inst = nc.gpsimd.indirect_dma_start(
            out=out_tile[:, j * C : (j + 1) * C], out_offset=None,
            in_=table_ap,
            in_offset=bass.IndirectOffsetOnAxis(ap=idx_ap[:, j : j + 1],
                                                axis=0))
        q = _swq_counter[0] % NSWQ
        _swq_counter[0] += 1
        if q:
            inst.ins.queue = f"qPoolDynamic{q}"


class _TC(tile.TileContext):
    """TileContext adapted to this container's neuronxcc build, which allows
    at most ONE sync wait per instruction: excess waits are hoisted onto
    injected same-engine nops (same semantics: the engine stalls at the nop
    until the sem condition holds, then proceeds to the instruction)."""

    def _spill_waits(self, inst, keep):
        si = getattr(inst, "sync_info", None)
        if si is None or si.on_wait is None or len(si.on_wait) <= keep:
            return
        waits = list(si.on_wait)
        extra, kept = waits[: len(waits) - keep], waits[len(waits) - keep :]
        del si.on_wait[:]
        si.on_wait.extend(kept)
        for wchunk in extra:
            nop = mybir.InstNoOp(
                name=self.nc.get_next_instruction_name(), ins=[], outs=[])
            nop.engine = inst.engine
            nop.sync_info = mybir.SyncInfo(on_wait=[wchunk], on_update=[])
            self._add_instruction(nop)

    def _commit_and_lower(self, inst, original_block, old_bb_map,
                          bb_to_exit_bb):
        eng = getattr(inst, "engine", None)
        if eng is not None and eng != mybir.EngineType.Unassigned:
            self._spill_waits(inst, keep=1)
        return super()._commit_and_lower(
            inst, original_block, old_bb_map, bb_to_exit_bb)

    def _drain_and_barrier(self, tick_clock, wait_clock):
        drain_inst = self.nc.sync.drain()
        wait_clock.add_sem_waits(
            drain_inst.ins, ScopedClock({None: tick_clock.global_clock})
        )
        si = drain_inst.ins.sync_info
        if si is not None and si.on_wait is not None and len(si.on_wait) > 0:
            waits = list(si.on_wait)
            del si.on_wait[:]
            for i in range(len(waits)):
                nop = self.nc.sync.nop(nofuse=True, hint="drain_wait_spill")
                nop.ins.sync_info = mybir.SyncInfo(
                    on_wait=waits[i : i + 1], on_update=[]
                )
        self.nc.all_engine_barrier()
        assert self.sems is not None
        popped = self.nc._tile_sem_poison_stack.pop()
        assert popped is self._sem_poison
        self.nc.clear_and_free_semaphores(list(self.sems.allocated().values()))
        self.nc.all_engine_barrier()


# ======================================================================
# host-side prep
# ======================================================================

def _col_layout(vals, ntiles, dtype, pad):
    """[n] -> [128, ntiles]; tile j's rows j*128..j*128+127 sit in column j,
    one per partition (matches the indirect-DMA offset layout)."""
    a = np.full(ntiles * 128, pad, dtype=dtype)
    a[: len(vals)] = vals
    return np.ascontiguousarray(a.reshape(ntiles, 128).T)


def _boundaries(sorted_locals, nbins, cap):
    """Greedy window boundaries over [0, nbins): each window gets <= cap rows
    in every stream and spans <= cap bins."""
    B = [0]
    ptrs = [0] * len(sorted_locals)
    while B[-1] < nbins:
        cur = B[-1]
        nxt = min(cur + cap, nbins)
        for si, t in enumerate(sorted_locals):
            i0 = ptrs[si]
            if i0 + cap < len(t):
                nxt = min(nxt, int(t[i0 + cap]))
        assert nxt > cur, "single bin exceeds window row capacity"
        for si, t in enumerate(sorted_locals):
            ptrs[si] = int(np.searchsorted(t, nxt, side="left"))
        B.append(nxt)
    return np.asarray(B, np.int64)


def _sorted_stream(targets, idx_a, idx_b):
    order = np.argsort(targets, kind="stable")
    return (
        np.asarray(targets)[order],
        np.asarray(idx_a)[order],
        np.asarray(idx_b)[order],
    )


def _edge_boundaries(R, s_streams, cores):
    """Per-core sorted local streams + window boundaries."""
    per_core_sorted, per_core_B = [], []
    for k in range(cores):
        lo, hi = R * k, R * (k + 1)
        loc = []
        for tg, ga, gb in s_streams:
            i0, i1 = np.searchsorted(tg, [lo, hi])
            loc.append((tg[i0:i1] - lo, ga[i0:i1], gb[i0:i1]))
        per_core_sorted.append(loc)
        per_core_B.append(_boundaries([l[0] for l in loc], R, PW))
    return per_core_sorted, per_core_B


def _edge_fill(W, R, per_core_sorted, per_core_B, p_idx, x_res, cores):
    """Build padded per-core arrays for one edge phase at uniform W."""
    out = []
    ns = len(per_core_sorted[0])
    for k in range(cores):
        lo = R * k
        B = per_core_B[k]
        nwin = len(B) - 1
        d = {}
        for si in range(ns):
            tg, ga, gb = per_core_sorted[k][si]
            gA = np.zeros((W * 2 * 128,), np.int64)
            gB = np.zeros((W * 2 * 128,), np.int64)
            tt = np.full((W * 2 * 128,), PAD_TGT, np.float32)
            for w in range(nwin):
                i0, i1 = np.searchsorted(tg, [B[w], B[w + 1]])
                n = i1 - i0
                assert n <= PW
                o = w * 2 * 128
                gA[o : o + n] = ga[i0:i1]
                gB[o : o + n] = gb[i0:i1]
                tt[o : o + n] = (tg[i0:i1] - B[w]).astype(np.float32)
            d[f"s{si}_ga"] = _col_layout(gA, W * 2, np.int32, 0)
            d[f"s{si}_gb"] = _col_layout(gB, W * 2, np.int32, 0)
            d[f"s{si}_tg"] = _col_layout(tt, W * 2, np.float32, PAD_TGT)
        pA = np.zeros((W * 2 * 128,), np.int64)
        pB = np.zeros((W * 2 * 128,), np.int64)
        xres = np.zeros((W * 256, C), np.float32)
        for w in range(nwin):
            wd = B[w + 1] - B[w]
            rows = np.arange(lo + B[w], lo + B[w + 1])
            o = w * 256
            pA[o : o + wd] = p_idx[0][rows]
            pB[o : o + wd] = p_idx[1][rows]
            xres[o : o + wd] = x_res[rows]
        d["p_ga"] = _col_layout(pA, W * 2, np.int32, 0)
        d["p_gb"] = _col_layout(pB, W * 2, np.int32, 0)
        # residual rows grouped: [g, p, u*256+j*128+c] <- row 256u+128j+p
        d["xres"] = np.ascontiguousarray(
            xres.reshape(W // GW, GW, 2, 128, C).transpose(0, 3, 1, 2, 4)
            .reshape(W // GW, 128, GW * 256))
        d["xt"] = np.ascontiguousarray(xres.T).astype(BF)  # [128c, W*256]
        out.append(d)
    return out


def _node_phase_prep(R0, streams, cores):
    """o0 prep: streams = [(sorted_targets, sorted_gidx)] x2 ; fixed 256-bin
    windows. Returns per-core arrays + uniform per-window tile counts."""
    NW = (R0 + PW - 1) // PW
    counts = np.zeros((2, NW, cores), np.int64)
    srt = []
    for k in range(cores):
        lo, hi = R0 * k, R0 * (k + 1)
        loc = []
        for si, (tg, ga) in enumerate(streams):
            i0, i1 = np.searchsorted(tg, [lo, hi])
            tl, gl = tg[i0:i1] - lo, ga[i0:i1]
            loc.append((tl, gl))
            for w in range(NW):
                j0, j1 = np.searchsorted(tl, [w * PW, (w + 1) * PW])
                counts[si, w, k] = j1 - j0
        srt.append(loc)
    T = []
    for si in range(2):
        tw = []
        for w in range(NW):
            need = int(counts[si, w].max())
            tiles = max(2, -(-need // 128))
            tiles += tiles % 2
            tw.append(tiles)
        tw[-1] += (-sum(tw)) % 8
        T.append(tw)
    out = []
    for k in range(cores):
        d = {}
        for si in range(2):
            tl, gl = srt[k][si]
            tot = sum(T[si])
            gA = np.zeros((tot * 128,), np.int64)
            tt = np.full((tot * 128,), PAD_TGT, np.float32)
            o = 0
            for w in range(NW):
                j0, j1 = np.searchsorted(tl, [w * PW, (w + 1) * PW])
                n = j1 - j0
                gA[o : o + n] = gl[j0:j1]
                tt[o : o + n] = (tl[j0:j1] - w * PW).astype(np.float32)
                o += T[si][w] * 128
            d[f"n{si}_ga"] = _col_layout(gA, tot, np.int32, 0)
            d[f"n{si}_tg"] = _col_layout(tt, tot, np.float32, PAD_TGT)
        out.append(d)
    return out, {"NW": NW, "T": T}


# ======================================================================
# device program
# ======================================================================

class _Consts:
    pass


def _load_consts(nc, tc, ctx, H, has_bias):
    cp = ctx.enter_context(tc.tile_pool(name="consts", bufs=1))
    K = _Consts()

    def _ld(nm, shape, dt):
        t = cp.tile(list(shape), dt, tag=nm)
        nc.sync.dma_start(t[:], H[nm].ap())
        setattr(K, nm, t)

    for nm in ("inner", "l111", "l222", "l211", "m0a", "m0b", "m1a", "m1b",
               "m2a", "m2b"):
        _ld(f"w_{nm}", (C, C), BF16)
    for nm in ("m0a_bv", "m1a_bv", "m2a_bv", "inner_bv"):
        _ld(nm, (C, 1), F32)
    if has_bias["a1"]:
        _ld("bias_a1v", (C, 1), F32)
    if has_bias["a2"]:
        _ld("bias_a2v", (C, 1), F32)
    _ld("iota_t", (C, PW), F32)
    _ld("idn_t", (C, C), BF16)
    if has_bias["rows"]:
        _ld("ones1_t", (1, C), BF16)
        for nm in ("inner_brow", "m0b_brow", "m1b_brow", "m2b_brow"):
            _ld(nm, (1, C), BF16)
    return K


def _bias_prefill(nc, K, ps, brow):
    """Fill [128, 256] PSUM (two row-major row-tiles) with bias rows."""
    for j in range(2):
        nc.tensor.matmul(out=ps[:, j * C : (j + 1) * C], lhsT=K.ones1_t[:],
                         rhs=brow[:], start=(j == 0) and True, stop=False,
                         skip_group_check=True)


def _emit_edge_phase(nc, tc, K, H, W, pfx, tables, wlin, wmlp_a, wmlp_b,
                     bias_a, mlp_a_bias, mlp_b_brow, has_inner_row, has_mb,
                     out_name):
    """One edge output (o1/o2). tables: {'p': ap, 's<i>': (apA, apB)}."""
    ns = len(wlin)
    snames = [f"s{i}" for i in range(ns)]
    with ExitStack() as phase:
        gp = phase.enter_context(tc.tile_pool(name=f"{pfx}_g", bufs=2))
        rp = phase.enter_context(tc.tile_pool(name=f"{pfx}_r", bufs=2))
        sp = phase.enter_context(tc.tile_pool(name=f"{pfx}_s", bufs=3))
        op = phase.enter_context(tc.tile_pool(name=f"{pfx}_o", bufs=2))
        ip = phase.enter_context(tc.tile_pool(name=f"{pfx}_i", bufs=1))
        tp_ps = phase.enter_context(
            tc.tile_pool(name=f"{pfx}_tp", bufs=2, space="PSUM"))
        ys_ps = phase.enter_context(
            tc.tile_pool(name=f"{pfx}_ys", bufs=2, space="PSUM"))
        win_ps = phase.enter_context(
            tc.tile_pool(name=f"{pfx}_win", bufs=2, space="PSUM"))
        mix_ps = phase.enter_context(
            tc.tile_pool(name=f"{pfx}_mix", bufs=2, space="PSUM"))

        NT = W * 2
        idx = {}
        for s in snames:
            for part, dt in (("ga", I32), ("gb", I32), ("tg", F32)):
                t = ip.tile([128, NT], dt, tag=f"{s}_{part}")
                nc.sync.dma_start(t[:], H[f"{pfx}_{s}_{part}"].ap())
                idx[f"{s}_{part}"] = t
        for part in ("ga", "gb"):
            t = ip.tile([128, NT], I32, tag=f"p_{part}")
            nc.sync.dma_start(t[:], H[f"{pfx}_p_{part}"].ap())
            idx[f"p_{part}"] = t

        xres_h = H[f"{pfx}_xres"].ap()
        xt_h = H[f"{pfx}_xt"].ap()
        out_h = H[out_name].ap()

        for g in range(W // GW):
            gbuf = {}
            for s in snames:
                ta, tb = tables[s]
                a = gp.tile([128, GW * 2 * C], BF16, tag=f"g_{s}a")
                bbuf = gp.tile([128, GW * 2 * C], BF16, tag=f"g_{s}b")
                _gather_tiles(nc, a, ta,
                              idx[f"{s}_ga"][:, g * 2 * GW : (g + 1) * 2 * GW],
                              2 * GW)
                _gather_tiles(nc, bbuf, tb,
                              idx[f"{s}_gb"][:, g * 2 * GW : (g + 1) * 2 * GW],
                              2 * GW)
                ssum = gp.tile([128, GW * 2 * C], BF16, tag=f"g_{s}sum")
                nc.vector.tensor_tensor(out=ssum[:], in0=a[:], in1=bbuf[:],
                                        op=ADD)
                gbuf[s] = ssum
            pa = gp.tile([128, GW * 2 * C], BF16, tag="g_pa")
            pb = gp.tile([128, GW * 2 * C], BF16, tag="g_pb")
            _gather_tiles(nc, pa, tables["p"],
                          idx["p_ga"][:, g * 2 * GW : (g + 1) * 2 * GW], 2 * GW)
            _gather_tiles(nc, pb, tables["p"],
                          idx["p_gb"][:, g * 2 * GW : (g + 1) * 2 * GW], 2 * GW)
            pbuf = gp.tile([128, GW * 2 * C], BF16, tag="g_psum")
            nc.vector.tensor_tensor(out=pbuf[:], in0=pa[:], in1=pb[:], op=ADD)

            xres_t = rp.tile([128, GW * 256], F32, tag="xres")
            nc.sync.dma_start(xres_t[:], xres_h[g])
            xt_t = rp.tile([128, GW * 256], BF16, tag="xt")
            nc.sync.dma_start(xt_t[:],
                              xt_h[:, g * GW * 256 : (g + 1) * GW * 256])
            ost = op.tile([128, GW * 256], F32, tag="ost")

            for u in range(GW):
                w = g * GW + u
                # ---------- positional stream (C-major) ----------
                ptp = tp_ps.tile([128, 256], BF16, tag="tp")
                for j in range(2):
                    nc.tensor.transpose(
                        out=ptp[:, j * C : (j + 1) * C],
                        in_=pbuf[:, (u * 2 + j) * C : (u * 2 + j + 1) * C],
                        identity=K.idn_t[:])
                ptT = sp.tile([128, 256], BF16, tag="ptT")
                nc.scalar.activation(ptT[:], ptp[:], COPY)
                y011 = mix_ps.tile([128, 256], F32, tag="mix")
                nc.tensor.matmul(out=y011[:], lhsT=K.w_inner[:], rhs=ptT[:],
                                 start=True, stop=True)
                y011s = sp.tile([128, 256], BF16, tag="y011s")
                nc.scalar.activation(y011s[:], y011[:], RELU,
                                     bias=K.inner_bv[:])

                # ---------- scatter streams (row-major) ----------
                lin = mix_ps.tile([128, 256], F32, tag="mix")
                for si, s in enumerate(snames):
                    stp = tp_ps.tile([128, 256], BF16, tag="tp")
                    for j in range(2):
                        nc.tensor.transpose(
                            out=stp[:, j * C : (j + 1) * C],
                            in_=gbuf[s][:, (u * 2 + j) * C : (u * 2 + j + 1) * C],
                            identity=K.idn_t[:])
                    stT = sp.tile([128, 256], BF16, tag="stT")
                    nc.scalar.activation(stT[:], stp[:], COPY)
                    ys = ys_ps.tile([128, 256], F32, tag="ys")
                    first = True
                    if has_inner_row:
                        _bias_prefill(nc, K, ys, K.inner_brow)
                        first = False
                    for j in range(2):
                        nc.tensor.matmul(
                            out=ys[:, j * C : (j + 1) * C],
                            lhsT=stT[:, j * C : (j + 1) * C],
                            rhs=K.w_inner[:], start=first, stop=True)
                    y_s = sp.tile([128, 256], BF16, tag="y_s")
                    nc.scalar.activation(y_s[:], ys[:], RELU)
                    oh = sp.tile([128, 512], BF16, tag="oh")
                    for j in range(2):
                        nc.vector.tensor_tensor(
                            out=oh[:, j * 256 : (j + 1) * 256],
                            in0=idx[f"{s}_tg"][:, w * 2 + j : w * 2 + j + 1]
                                .to_broadcast([128, 256]),
                            in1=K.iota_t[:], op=ISEQ)
                    win = win_ps.tile([128, 256], F32, tag="win")
                    for j in range(2):
                        nc.tensor.matmul(
                            out=win[:], lhsT=y_s[:, j * C : (j + 1) * C],
                            rhs=oh[:, j * 256 : (j + 1) * 256],
                            start=(j == 0), stop=(j == 1))
                    segT = sp.tile([128, 256], BF16, tag="segT")
                    nc.scalar.activation(segT[:], win[:], COPY)
                    nc.tensor.matmul(out=lin[:], lhsT=wlin[si][:], rhs=segT[:],
                                     start=(si == 0), stop=(si == ns - 1))

                # ---------- assembly + MLP ----------
                zt = sp.tile([128, 256], BF16, tag="zt")
                nc.vector.tensor_tensor(
                    out=zt[:], in0=xt_t[:, u * 256 : (u + 1) * 256],
                    in1=y011s[:], op=ADD)
                nc.vector.tensor_tensor(out=zt[:], in0=zt[:], in1=lin[:],
                                        op=ADD)
                if bias_a is not None:
                    nc.vector.tensor_tensor(
                        out=zt[:], in0=zt[:],
                        in1=bias_a[:].to_broadcast([128, 256]), op=ADD)
                hT = mix_ps.tile([128, 256], F32, tag="mix")
                nc.tensor.matmul(out=hT[:], lhsT=wmlp_a[:], rhs=zt[:],
                                 start=True, stop=True)
                hTs = sp.tile([128, 256], BF16, tag="hTs")
                nc.scalar.activation(hTs[:], hT[:], RELU, bias=mlp_a_bias[:])
                omm = mix_ps.tile([128, 256], F32, tag="mix")
                first = True
                if has_mb:
                    _bias_prefill(nc, K, omm, mlp_b_brow)
                    first = False
                for j in range(2):
                    nc.tensor.matmul(
                        out=omm[:, j * C : (j + 1) * C],
                        lhsT=hTs[:, j * C : (j + 1) * C], rhs=wmlp_b[:],
                        start=first, stop=True)
                nc.vector.tensor_tensor(
                    out=ost[:, u * 256 : (u + 1) * 256], in0=omm[:],
                    in1=xres_t[:, u * 256 : (u + 1) * 256], op=ADD)
            nc.sync.dma_start(out_h[g], ost[:])


def _emit_node_phase(nc, tc, K, H, meta0, tables, has_inner_row, has_m0b):
    NW, T = meta0["NW"], meta0["T"]
    with ExitStack() as phase:
        gp = phase.enter_context(tc.tile_pool(name="n_g", bufs=2))
        sp = phase.enter_context(tc.tile_pool(name="n_s", bufs=3))
        accp = phase.enter_context(tc.tile_pool(name="n_acc", bufs=1))
        ip = phase.enter_context(tc.tile_pool(name="n_i", bufs=1))
        op = phase.enter_context(tc.tile_pool(name="n_o", bufs=2))
        tp_ps = phase.enter_context(
            tc.tile_pool(name="n_tp", bufs=2, space="PSUM"))
        ys_ps = phase.enter_context(
            tc.tile_pool(name="n_ys", bufs=2, space="PSUM"))
        win_ps = phase.enter_context(
            tc.tile_pool(name="n_win", bufs=2, space="PSUM"))
        mix_ps = phase.enter_context(
            tc.tile_pool(name="n_mix", bufs=2, space="PSUM"))

        NB = NW * PW
        a0 = accp.tile([128, NB], F32, tag="a0sb")
        nc.scalar.memzero(a0[:])

        idx = {}
        for si in range(2):
            tot = sum(T[si])
            t = ip.tile([128, tot], I32, tag=f"n{si}_ga")
            nc.sync.dma_start(t[:], H[f"n{si}_ga"].ap())
            idx[f"{si}_ga"] = t
            t2 = ip.tile([128, tot], F32, tag=f"n{si}_tg")
            nc.sync.dma_start(t2[:], H[f"n{si}_tg"].ap())
            idx[f"{si}_tg"] = t2

        for si in range(2):
            base = 0
            ranges = []
            for w in range(NW):
                ranges.append((base, base + T[si][w]))
                base += T[si][w]
            cur = {"g": -1, "t": None}

            def _gtile(ti, si=si, cur=cur):
                gidx = ti // 8
                if cur["g"] != gidx:
                    a = gp.tile([128, 8 * C], BF16, tag=f"ng{si}")
                    _gather_tiles(nc, a, tables[si],
                                  idx[f"{si}_ga"][:, gidx * 8 : (gidx + 1) * 8],
                                  8)
                    cur["g"], cur["t"] = gidx, a
                return cur["t"], ti % 8

            for w in range(NW):
                t0, t1 = ranges[w]
                npr = (t1 - t0) // 2
                win = win_ps.tile([128, 256], F32, tag="nwin")
                for pr in range(npr):
                    tA, tB = t0 + 2 * pr, t0 + 2 * pr + 1
                    stp = tp_ps.tile([128, 256], BF16, tag="ntp")
                    for j, ti in enumerate((tA, tB)):
                        gt, off = _gtile(ti)
                        nc.tensor.transpose(
                            out=stp[:, j * C : (j + 1) * C],
                            in_=gt[:, off * C : (off + 1) * C],
                            identity=K.idn_t[:])
                    stT = sp.tile([128, 256], BF16, tag="nstT")
                    nc.scalar.activation(stT[:], stp[:], COPY)
                    ys = ys_ps.tile([128, 256], F32, tag="nys")
                    first = True
                    if has_inner_row:
                        _bias_prefill(nc, K, ys, K.inner_brow)
                        first = False
                    for j in range(2):
                        nc.tensor.matmul(
                            out=ys[:, j * C : (j + 1) * C],
                            lhsT=stT[:, j * C : (j + 1) * C],
                            rhs=K.w_inner[:], start=first, stop=True)
                    y_s = sp.tile([128, 256], BF16, tag="ny_s")
                    nc.scalar.activation(y_s[:], ys[:], RELU)
                    oh = sp.tile([128, 512], BF16, tag="noh")
                    for j, ti in enumerate((tA, tB)):
                        nc.vector.tensor_tensor(
                            out=oh[:, j * 256 : (j + 1) * 256],
                            in0=idx[f"{si}_tg"][:, ti : ti + 1]
                                .to_broadcast([128, 256]),
                            in1=K.iota_t[:], op=ISEQ)
                    for j in range(2):
                        nc.tensor.matmul(
                            out=win[:], lhsT=y_s[:, j * C : (j + 1) * C],
                            rhs=oh[:, j * 256 : (j + 1) * 256],
                            start=(pr == 0 and j == 0),
                            stop=(pr == npr - 1 and j == 1))
                nc.vector.tensor_tensor(
                    out=a0[:, w * PW : (w + 1) * PW],
                    in0=a0[:, w * PW : (w + 1) * PW], in1=win[:], op=ADD)

        x0t = ip.tile([128, NB], BF16, tag="x0t")
        nc.sync.dma_start(x0t[:], H["x0t_loc"].ap())
        x0r_h = H["x0r_loc"].ap()
        o0_h = H["o0w"].ap()
        for w in range(NW):
            x0r = op.tile([128, 256], F32, tag="x0r")
            nc.sync.dma_start(x0r[:], x0r_h[w])
            zt = sp.tile([128, 256], BF16, tag="nzt")
            nc.vector.tensor_tensor(
                out=zt[:], in0=x0t[:, w * PW : (w + 1) * PW],
                in1=a0[:, w * PW : (w + 1) * PW], op=ADD)
            hT = mix_ps.tile([128, 256], F32, tag="nmix")
            nc.tensor.matmul(out=hT[:], lhsT=K.w_m0a[:], rhs=zt[:],
                             start=True, stop=True)
            hTs = sp.tile([128, 256], BF16, tag="nhTs")
            nc.scalar.activation(hTs[:], hT[:], RELU, bias=K.m0a_bv[:])
            omm = mix_ps.tile([128, 256], F32, tag="nmix")
            first = True
            if has_m0b:
                _bias_prefill(nc, K, omm, K.m0b_brow)
                first = False
            for j in range(2):
                nc.tensor.matmul(
                    out=omm[:, j * C : (j + 1) * C],
                    lhsT=hTs[:, j * C : (j + 1) * C], rhs=K.w_m0b[:],
                    start=first, stop=True)
            ot = op.tile([128, 256], F32, tag="not")
            nc.vector.tensor_tensor(out=ot[:], in0=omm[:], in1=x0r[:], op=ADD)
            nc.sync.dma_start(o0_h[w], ot[:])


# ======================================================================
# top level
# ======================================================================

def _build_and_run(inputs, N0, E1, E2, cores=NCORES, run=True):
    R1, R2, R0 = E1 // cores, E2 // cores, N0 // cores
    f = lambda nm: np.asarray(inputs[nm], np.float32)
    ii = lambda nm: np.asarray(inputs[nm], np.int64)
    x0, x1, x2 = f("x0"), f("x1"), f("x2")
    ei1, ei2 = ii("ei1"), ii("ei2")
    t111, t222, t112 = ii("tri111"), ii("tri222"), ii("tri112")
    inv1inv = np.argsort(ii("inv1"))

    # ---------------- host prep ----------------
    o1_streams = [
        _sorted_stream(t111[0], t111[1], t111[2]),          # -> l111 (x1,x1)
        _sorted_stream(t112[0], t112[1], t112[2]),          # u  -> l211 (x1,x2)
        _sorted_stream(inv1inv[t112[0]], t112[1], t112[2]), # v  -> l211 (x1,x2)
    ]
    o2_streams = [
        _sorted_stream(t222[0], t222[1], t222[2]),          # -> l222 (x2,x2)
        _sorted_stream(t112[2], t112[0], t112[1]),          # -> l211 (x1,x1)
    ]
    srt1, B1 = _edge_boundaries(R1, o1_streams, cores)
    srt2, B2 = _edge_boundaries(R2, o2_streams, cores)
    W = max(max(len(b) - 1 for b in B1), max(len(b) - 1 for b in B2))
    W = (W + GW - 1) // GW * GW
    e1_pc = _edge_fill(W, R1, srt1, B1, (ei1[0], ei1[1]), x1, cores)
    e2_pc = _edge_fill(W, R2, srt2, B2, (ei2[0], ei2[1]), x2, cores)

    sn1 = _sorted_stream(ei1[0], np.arange(E1), np.arange(E1))
    sn2 = _sorted_stream(ei2[0], np.arange(E2), np.arange(E2))
    n_pc, m0 = _node_phase_prep(R0, [(sn1[0], sn1[1]), (sn2[0], sn2[1])],
                                cores)
    NW = m0["NW"]

    wT = {nm: np.ascontiguousarray(f(nm + "_W").T).astype(BF)
          for nm in ("inner", "l111", "l222", "l211",
                     "m0a", "m0b", "m1a", "m1b", "m2a", "m2b")}
    b = {nm: f(nm + "_b") for nm in ("inner", "l111", "l222", "l211",
                                     "m0a", "m0b", "m1a", "m1b", "m2a", "m2b")}
    has_bias = {
        "inner_row": bool(np.any(b["inner"])),
        "a1": bool(np.any(b["l111"] + b["l211"])),
        "a2": bool(np.any(b["l222"] + b["l211"])),
        "m0b": bool(np.any(b["m0b"])),
        "m1b": bool(np.any(b["m1b"])),
        "m2b": bool(np.any(b["m2b"])),
    }
    has_bias["rows"] = (has_bias["inner_row"] or has_bias["m0b"]
                        or has_bias["m1b"] or has_bias["m2b"])

    # ---------------- program ----------------
    nc = bass.Bass("TRN2", target_bir_lowering=False, debug=False,
                   num_swdge_queues=NSWQ)
    H = {}

    def din(nm, shp, dt):
        H[nm] = nc.dram_tensor(nm, list(shp), dt, kind="ExternalInput")

    def dout(nm, shp, dt):
        H[nm] = nc.dram_tensor(nm, list(shp), dt, kind="ExternalOutput")

    din("x0_bf", (N0, C), BF16)
    din("x1_bf", (E1, C), BF16)
    din("x2_bf", (E2, C), BF16)
    for nm in wT:
        din(f"w_{nm}", (C, C), BF16)
    for nm in ("m0a_bv", "m1a_bv", "m2a_bv", "inner_bv"):
        din(nm, (C, 1), F32)
    if has_bias["a1"]:
        din("bias_a1v", (C, 1), F32)
    if has_bias["a2"]:
        din("bias_a2v", (C, 1), F32)
    din("iota_t", (C, PW), F32)
    din("idn_t", (C, C), BF16)
    if has_bias["rows"]:
        din("ones1_t", (1, C), BF16)
        for nm in ("inner_brow", "m0b_brow", "m1b_brow", "m2b_brow"):
            din(nm, (1, C), BF16)
    for pfx, nstream in (("e1", 3), ("e2", 2)):
        for i in range(nstream):
            din(f"{pfx}_s{i}_ga", (128, W * 2), I32)
            din(f"{pfx}_s{i}_gb", (128, W * 2), I32)
            din(f"{pfx}_s{i}_tg", (128, W * 2), F32)
        din(f"{pfx}_p_ga", (128, W * 2), I32)
        din(f"{pfx}_p_gb", (128, W * 2), I32)
        din(f"{pfx}_xres", (W // GW, 128, GW * 256), F32)
        din(f"{pfx}_xt", (128, W * 256), BF16)
    for si in range(2):
        tot = sum(m0["T"][si])
        din(f"n{si}_ga", (128, tot), I32)
        din(f"n{si}_tg", (128, tot), F32)
    din("x0t_loc", (128, NW * PW), BF16)
    din("x0r_loc", (NW, 128, 256), F32)
    dout("o1w", (W // GW, 128, GW * 256), F32)
    dout("o2w", (W // GW, 128, GW * 256), F32)
    dout("o0w", (NW, 128, 256), F32)

    x0_t, x1_t, x2_t = H["x0_bf"].ap(), H["x1_bf"].ap(), H["x2_bf"].ap()
    with _TC(nc) as tc:
        with ExitStack() as ctx:
            K = _load_consts(nc, tc, ctx, H, has_bias)
            _emit_edge_phase(
                nc, tc, K, H, W, "e1",
                {"p": x0_t, "s0": (x1_t, x1_t), "s1": (x1_t, x2_t),
                 "s2": (x1_t, x2_t)},
                [K.w_l111, K.w_l211, K.w_l211], K.w_m1a, K.w_m1b,
                K.bias_a1v if has_bias["a1"] else None, K.m1a_bv,
                K.m1b_brow if has_bias["rows"] else None,
                has_bias["inner_row"], has_bias["m1b"], "o1w")
            _emit_edge_phase(
                nc, tc, K, H, W, "e2",
                {"p": x0_t, "s0": (x2_t, x2_t), "s1": (x1_t, x1_t)},
                [K.w_l222, K.w_l211], K.w_m2a, K.w_m2b,
                K.bias_a2v if has_bias["a2"] else None, K.m2a_bv,
                K.m2b_brow if has_bias["rows"] else None,
                has_bias["inner_row"], has_bias["m2b"], "o2w")
            _emit_node_phase(nc, tc, K, H, m0, [x1_t, x2_t],
                             has_bias["inner_row"], has_bias["m0b"])

    # ---------------- in_maps ----------------
    iota = np.broadcast_to(np.arange(PW, dtype=np.float32), (C, PW)).copy()
    rep = {
        "x0_bf": x0.astype(BF), "x1_bf": x1.astype(BF), "x2_bf": x2.astype(BF),
        "iota_t": iota, "idn_t": np.eye(C, dtype=BF),
        "m0a_bv": b["m0a"].reshape(C, 1), "m1a_bv": b["m1a"].reshape(C, 1),
        "m2a_bv": b["m2a"].reshape(C, 1), "inner_bv": b["inner"].reshape(C, 1),
    }
    for nm, v in wT.items():
        rep[f"w_{nm}"] = v
    if has_bias["a1"]:
        rep["bias_a1v"] = (b["l111"] + b["l211"]).reshape(C, 1)
    if has_bias["a2"]:
        rep["bias_a2v"] = (b["l222"] + b["l211"]).reshape(C, 1)
    if has_bias["rows"]:
        rep["ones1_t"] = np.ones((1, C), BF)
        for nm in ("inner", "m0b", "m1b", "m2b"):
            rep[f"{nm}_brow"] = b[nm].reshape(1, C).astype(BF)

    in_maps = []
    for k in range(cores):
        m = dict(rep)
        for key, v in e1_pc[k].items():
            m[f"e1_{key}"] = v
        for key, v in e2_pc[k].items():
            m[f"e2_{key}"] = v
        m.update(n_pc[k])
        lo = R0 * k
        x0l = np.zeros((NW * PW, C), np.float32)
        x0l[:R0] = x0[lo : lo + R0]
        m["x0t_loc"] = np.ascontiguousarray(x0l.T).astype(BF)
        m["x0r_loc"] = np.ascontiguousarray(
            x0l.reshape(NW, 2, 128, C).transpose(0, 2, 1, 3)
            .reshape(NW, 128, 256))
        in_maps.append(m)

    if not run:
        return nc, in_maps, (W, NW, B1, B2, R0, R1, R2)

    res = run_bass_kernel_spmd(nc, in_maps, list(range(cores)))
    return _assemble(res.results, N0, E1, E2, cores, W, NW, B1, B2,
                     R0, R1, R2)


def _assemble(results, N0, E1, E2, cores, W, NW, B1, B2, R0, R1, R2):
    o0 = np.empty((N0, C), np.float32)
    o1 = np.empty((E1, C), np.float32)
    o2 = np.empty((E2, C), np.float32)
    for k in range(cores):
        r = results[k]
        ow = (r["o0w"].reshape(NW, 128, 2, 128).transpose(0, 2, 1, 3)
              .reshape(-1, C))
        o0[R0 * k : R0 * (k + 1)] = ow[:R0]
        for nm, out, B, R in (("o1w", o1, B1[k], R1), ("o2w", o2, B2[k], R2)):
            ow = (r[nm].reshape(W // GW, 128, GW, 2, 128)
                  .transpose(0, 2, 3, 1, 4).reshape(W * 256, C))
            lo = R * k
            for w in range(len(B) - 1):
                wd = B[w + 1] - B[w]
                out[lo + B[w] : lo + B[w + 1]] = ow[w * 256 : w * 256 + wd]
    return o0, o1, o2


def kernel(**inputs):
    return _build_and_run(inputs, N0=20000, E1=500000, E2=500000)


# ======================================================================
# device-time measurement (test harness use): run the program with
# device-resident inputs so repeated executions measure device time +
# dispatch only, not host->device upload.
# ======================================================================

def kernel_timed(inputs, iters=6, N0=20000, E1=500000, E2=500000):
    import time

    import jax
    import jax.numpy as jnp
    from jax.experimental.shard_map import shard_map
    from jax.sharding import Mesh, NamedSharding, PartitionSpec

    from concourse import bass2jax
    from concourse.bass2jax import (_bass_exec_p, install_neuronx_cc_hook,
                                    partition_id_tensor)

    nc, in_maps, _meta = _build_and_run(inputs, N0=N0, E1=E1, E2=E2,
                                        run=False)
    install_neuronx_cc_hook()
    n_cores = len(in_maps)
    partition_name = (nc.partition_id_tensor.name
                      if nc.partition_id_tensor else None)
    in_names, out_names, out_avals, zero_outs = [], [], [], []
    for alloc in nc.m.functions[0].allocations:
        if not isinstance(alloc, mybir.MemoryLocationSet):
            continue
        name = alloc.memorylocations[0].name
        if alloc.kind == "ExternalInput":
            if name != partition_name:
                in_names.append(name)
        elif alloc.kind == "ExternalOutput":
            out_names.append(name)
            shape = tuple(alloc.tensor_shape)
            dtype = mybir.dt.np(alloc.dtype)
            out_avals.append(jax.core.ShapedArray(shape, dtype))
            zero_outs.append((shape, dtype))
    n_params = len(in_names)
    all_in_names = list(in_names) + out_names
    if partition_name is not None:
        all_in_names.append(partition_name)

    def _body(*args):
        operands = list(args)
        if partition_name is not None:
            operands.append(partition_id_tensor())
        outs = _bass_exec_p.bind(
            *operands, out_avals=tuple(out_avals),
            in_names=tuple(all_in_names), out_names=tuple(out_names),
            lowering_input_output_aliases=(), sim_require_finite=True,
            sim_require_nnan=True, nc=nc)
        return tuple(outs)

    devices = jax.devices()[:n_cores]
    mesh = Mesh(np.asarray(devices), ("core",))
    spec = NamedSharding(mesh, PartitionSpec("core"))
    n_outs = len(out_avals)
    donate = tuple(range(n_params, n_params + n_outs))
    sharded = jax.jit(
        shard_map(_body, mesh=mesh,
                  in_specs=(PartitionSpec("core"),) * (n_params + n_outs),
                  out_specs=(PartitionSpec("core"),) * n_outs,
                  check_rep=False),
        donate_argnums=donate, keep_unused=True)

    dev_in = []
    for i, nm in enumerate(in_names):
        cat = np.concatenate([np.asarray(m[nm]) for m in in_maps], axis=0)
        dev_in.append(jax.device_put(cat, spec))

    mkzeros = jax.jit(
        lambda: tuple(jnp.zeros((n_cores * s[0], *s[1:]), d)
                      for s, d in zero_outs),
        out_shardings=(spec,) * n_outs)

    # warm-up (compiles)
    z = mkzeros()
    jax.block_until_ready(sharded(*dev_in, *z))
    # time zeros generation alone
    t0 = time.time()
    zs = [mkzeros() for _ in range(iters)]
    jax.block_until_ready(zs)
    t_zero = (time.time() - t0) / iters
    ts = []
    for i in range(iters):
        z = mkzeros()
        jax.block_until_ready(z)
        t0 = time.time()
        out = sharded(*dev_in, *z)
        jax.block_until_ready(out)
        ts.append(time.time() - t0)
    t_best = min(ts)
    print(f"exec wall per call: best={t_best*1e3:.2f} ms  all="
          f"{[f'{t*1e3:.1f}' for t in ts]} (zero-gen {t_zero*1e3:.2f} ms)")

    W, NW, B1, B2, R0, R1, R2 = _meta
    results = [
        {nm: np.asarray(out[i]).reshape(n_cores, *out_avals[i].shape)[c]
         for i, nm in enumerate(out_names)}
        for c in range(n_cores)
    ]
    outs = _assemble(results, N0, E1, E2, n_cores, W, NW, B1, B2, R0, R1, R2)
    return outs, t_best * 1e9
